# revision 16
# baseline (speedup 1.0000x reference)
"""Trainium2 Bass kernel for the DiscMaker mkaarma/controller scan.

Math per step t (per batch element b):
    ns    = tanh(x_t @ Wx[j] + kstate @ Wh[j])          j=0..2   [B,3,S]
    enc   = tanh(x_t @ We + kstate @ Ue)                         [B,E]
    cst   = tanh([enc, err] @ Wi + cst @ Whc)                    [B,H]
    out   = cst @ Wo                                             [B,4]
    gate  = softmax(out[:, :3] @ Wd + bd) ; theta = sigmoid(out[:, 3])
    gate  = gate*theta + gate_prev*(1-theta)
    kstate= sum_j gate[:,j] * ns[:,j,:] ; pred = kstate[:,-1] ; err = pred - y_t

Device design (per core, batch shard b=32, feature-on-partition).  The scan is
latency-bound: the serial spine per step is
    gate -> gb broadcast (PE) -> G = ns*gate (DVE) -> Ue/Wi ladder (PE/ACT)
    -> head (PE) -> exp (ACT) -> softmax blend (DVE) -> gate'
so the kernel optimizes the spine:
  - kstate never materialized: carry G[s,(j,b)] = gate[j,b]*ns[s,j,b]; all
    kstate consumers contract G with ONE matmul each whose PSUM out AP repeats
    over j (stride-0) so the 3 j-slices accumulate via has_written bits.
  - err enters the controller through PE only: Wib embedded in row 0 of a
    K=128 weight contracts G (row 0 = pred feature) straight into the
    controller PSUM; -Wib*y_t is a rank-1 matmul off the critical path.
  - gate algebra: exp (no accumulator read), then DVE reduce -> recip -> one
    2-wide packed stt ds=[e*r0 - g | e*r0 + g] -> gn = th2*dd + ss.
    theta via sigmoid(z) = (1+tanh(z/2))/2 keeps the {tanh, exp} ACT table.
  - gate head folded: Wfold = [Wo[:, :3]@Wd, 0.5*Wo[:,3]].
  - software pipelining: whc/negy/input/bdext matmuls for step t+1 are emitted
    into step t's gate-phase PE idle windows; junk matmuls fill the remaining
    PE gaps so the PE p-state stays at 2.4 GHz and the SBUF-access pipeline
    stays primed (first-matmul-after-idle costs ~185ns otherwise).
  - preds come from G[0,:] which is DMA'd out once; host sums over j.
"""

import os
import sys

import numpy as np

sys.path.insert(0, "/opt/trn_rl_repo")

import concourse.bass as bass  # noqa: E402
import concourse.tile as tile  # noqa: E402
from concourse import bacc, mybir  # noqa: E402

F16 = mybir.dt.float16
F32 = mybir.dt.float32
AF = mybir.ActivationFunctionType
ALU = mybir.AluOpType

B, T, D_IN, S, E, H, NOUT = 256, 512, 64, 128, 128, 256, 4
NCORES = 8
BC = B // NCORES  # 32 batch per core


def _junk_cols(env, default):
    v = os.environ.get(env)
    if v is None or v == "":
        return default
    if v.strip() in ("none", "-"):
        return []
    return [int(c) for c in v.split(",") if c]


FUSE_J = bool(int(os.environ.get("TRN_FUSE_J", "1")))
ERR_PE = bool(int(os.environ.get("TRN_ERR_PE", "1")))
DS_PACK = bool(int(os.environ.get("TRN_DS_PACK", "1")))
JUNK1 = _junk_cols("TRN_JUNK1", [])
JUNK2 = _junk_cols("TRN_JUNK2", [128, 64])
JUNK3 = _junk_cols("TRN_JUNK3", [256, 256])
JUNK4 = _junk_cols("TRN_JUNK4", [256, 128, 64])


def build_program(T_steps=T):
    nc = bacc.Bacc(
        "TRN2", target_bir_lowering=False, debug=False, enable_asserts=False
    )
    Tn = T_steps

    def din(name, shape, dt=F16):
        return nc.dram_tensor(name, shape, dt, kind="ExternalInput").ap()

    xw = din("xw", [S, Tn * BC])           # x[b,t,d] -> [d, t*32+b], d padded->128
    negy = din("negy", [1, Tn * BC])       # -y[b,t]  -> [1, t*32+b]
    gf20 = din("gf20", [S, 32])            # 2*gate0 in rows 0:32 cols 0:3, else 0
    whcat = din("whcat", [S, 3 * S])       # Wh[j][s_in, s_out] stacked on free
    wxcat = din("wxcat", [S, 3 * S])       # Wx[j] zero-padded rows 64:128
    ue = din("ue", [S, E])
    we = din("we", [S, E])                 # We zero-padded rows 64:128
    wit0 = din("wit0", [E, S])             # Wi[0:128] cols 0:128
    wit1 = din("wit1", [E, S])             # Wi[0:128] cols 128:256
    wib = din("wib", [1, H])               # Wi[128]
    wibr0 = din("wibr0", [S, H])           # Wi[128] embedded in row 0, else 0
    whc00 = din("whc00", [S, S])           # Whc[0:128, 0:128]
    whc01 = din("whc01", [S, S])           # Whc[0:128, 128:256]
    whc10 = din("whc10", [S, S])           # Whc[128:256, 0:128]
    whc11 = din("whc11", [S, S])           # Whc[128:256, 128:256]
    wfold = din("wfold", [S, 8])           # [F[0:128] | F[128:256]], F=[Wo@Wd_ext]
    bdext = din("bdext", [1, 4])           # [bd, 0]
    eye32 = din("eye32", [S, BC])          # 0.5*I_32 in rows 0:32, zeros below
    ones132 = din("ones132", [1, S])       # 1.0 in cols 0:32, 0 elsewhere

    g127o = nc.dram_tensor("g127", [1, Tn * 96], F16, kind="ExternalOutput").ap()

    with tile.TileContext(nc) as tc:
        import contextlib
        stk = contextlib.ExitStack()
        persist = stk.enter_context(tc.tile_pool(name="persist", bufs=1))

        def ptile(shape, dtype, name):
            return persist.tile(shape, dtype, name=name, tag=name)

        # ---- persistent SBUF ----
        s_xw = ptile([S, Tn * BC], F16, "s_xw")
        s_negy = ptile([1, Tn * BC], F16, "s_negy")
        s_whcat = ptile([S, 3 * S], F16, "s_whcat")
        s_wxcat = ptile([S, 3 * S], F16, "s_wxcat")
        s_ue = ptile([S, E], F16, "s_ue")
        s_we = ptile([S, E], F16, "s_we")
        s_wit = [ptile([E, S], F16, "s_wit0"), ptile([E, S], F16, "s_wit1")]
        s_wib = ptile([1, H], F16, "s_wib")
        s_wibr0 = ptile([S, H], F16, "s_wibr0")
        s_whc = [[ptile([S, S], F16, "s_whc00"), ptile([S, S], F16, "s_whc01")],
                 [ptile([S, S], F16, "s_whc10"), ptile([S, S], F16, "s_whc11")]]
        s_wfold = ptile([S, 8], F16, "s_wfold")
        s_bdext = ptile([1, 4], F16, "s_bdext")
        s_eye = ptile([S, BC], F16, "s_eye")
        s_ones132 = ptile([1, S], F16, "s_ones132")
        s_gall = ptile([S, Tn * 96], F16, "s_gall")
        s_cst = ptile([S, 5 * BC], F16, "s_cst")
        gf2a = ptile([S, 32], F16, "gf2a")
        gf2b = ptile([S, 32], F16, "gf2b")
        s_z1 = ptile([S, 1], F16, "s_z1")
        chalf = ptile([BC, 6], F32, "chalf")
        hgp2a = ptile([BC, 6], F32, "hgp2a")
        hgp2b = ptile([BC, 6], F32, "hgp2b")

        for dst, src in [
            (s_xw, xw), (s_negy, negy), (s_whcat, whcat), (s_wxcat, wxcat),
            (s_ue, ue), (s_we, we), (s_wit[0], wit0), (s_wit[1], wit1),
            (s_wib, wib), (s_wibr0, wibr0),
            (s_whc[0][0], whc00), (s_whc[0][1], whc01),
            (s_whc[1][0], whc10), (s_whc[1][1], whc11), (s_wfold, wfold),
            (s_bdext, bdext), (s_eye, eye32), (s_ones132, ones132),
            (gf2a, gf20),
        ]:
            nc.sync.dma_start(out=dst[:], in_=src)
        nc.vector.memset(gf2b[:], 0.0)
        nc.vector.memset(s_z1[:], 0.0)
        nc.vector.memset(s_cst[:, 2 * BC:5 * BC], 0.0)
        nc.vector.memset(chalf[:, 0:3], -0.5)
        nc.vector.memset(chalf[:, 3:6], 0.5)
        # hgp2 = [-gate | +gate] = gf2 * [-0.5 | +0.5]   (gf2 carries 2*gate)
        c3 = chalf[:].rearrange("p (a b) -> p a b", a=2)
        nc.vector.tensor_tensor(
            hgp2a[:].rearrange("p (a b) -> p a b", a=2),
            gf2a[0:BC, 0:3].unsqueeze(1).broadcast_to([BC, 2, 3]),
            c3, ALU.mult)

        # ---- pools ----
        # PSUM is 8 banks of 2KB/partition; zero regions (start=True scope)
        # are bank-sized, so each bank gets exactly ONE start=True per step
        # (the first write); everything else store-on-first-touch/accumulates.
        # The tiny gate-head tile g shares pEnc's bank (cols 32:36).
        pEnc = stk.enter_context(tc.tile_pool(name="pEnc", bufs=2, space="PSUM"))
        pNs = stk.enter_context(tc.tile_pool(name="pNs", bufs=2, space="PSUM"))
        pB = stk.enter_context(tc.tile_pool(name="pB", bufs=2, space="PSUM"))
        pGB = stk.enter_context(tc.tile_pool(name="pGB", bufs=1, space="PSUM"))
        pJ = stk.enter_context(tc.tile_pool(name="pJ", bufs=1, space="PSUM"))
        wk = stk.enter_context(tc.tile_pool(name="wk", bufs=3))

        ts = bass.ts
        gf2 = [gf2a, gf2b]
        hgp2 = [hgp2a, hgp2b]
        mm = nc.tensor.matmul

        def junk(cols_list, rhs1=None):
            # dependency-gated junk: rhs1 (a [K,1] fp16 SBUF AP) delays
            # readiness until its producer finishes, so the junk fills a
            # specific pipeline gap instead of greedily running early
            for c in cols_list:
                jt = pJ.tile([S, 512], F32, tag="junk")
                if rhs1 is None:
                    lhsT, rhs = s_whcat[:, 0:S], s_whcat[:, 0:c]
                else:
                    kp = rhs1.partition_size()
                    lhsT = s_whcat[0:kp, 0:S]
                    rhs = rhs1.broadcast_to([kp, c])
                mm(jt[:, 0:c], lhsT, rhs,
                   start=True, stop=True, skip_group_check=True)

        def bj(ap):  # [S, n] -> [S, 3, n] with stride-0 j (PSUM accumulate)
            return ap.unsqueeze(1).broadcast_to([ap.shape[0], 3, ap.shape[1]])

        # ---- prologue: step-0 input matmuls ----
        xt0 = s_xw[:, 0:BC]
        eg_cur = pEnc.tile([S, 512], F32, tag="a_enc")
        enc_cur = eg_cur[:, 0:BC]
        g_cur = eg_cur[:, BC:BC + 4]
        mm(enc_cur, s_we[:], xt0, start=True, stop=True)
        mm(g_cur, s_ones132[:], s_bdext[:], start=False, stop=False)
        ns_cur = pNs.tile([S, 512], F32, name="a_ns", tag="a_ns")[:, 0:96]
        for k in range(3):
            mm(ns_cur[:, ts(k, BC)], s_wxcat[:, ts(k, S)], xt0,
               start=(k == 0), stop=(k == 2))
        b_cur = pB.tile([S, 512], F32, name="b_", tag="b_")[:, 0:2 * BC]

        for t in range(Tn):
            first = (t == 0)
            last = (t == Tn - 1)
            gprev = None if first else s_gall[:, ts(t - 1, 96)]
            gn = gf2[(t + 1) % 2]

            # ---- [PE] G-dependent accumulations (spine head) ----
            if not first:
                if FUSE_J:
                    mm(bj(enc_cur), s_ue[:], gprev, start=False, stop=False)
                else:
                    for j in range(3):
                        mm(enc_cur, s_ue[:], gprev[:, ts(j, BC)],
                           start=False, stop=False)
                if ERR_PE:
                    # Wib x pred: row-0-embedded weight contracts G (j-summed)
                    for h in range(2):
                        mm(bj(b_cur[:, ts(h, BC)]), s_wibr0[:, ts(h, S)],
                           gprev, start=False, stop=False)
                else:
                    red = wk.tile([1, BC], F32, tag="red")
                    src3 = gprev[0:1, :].rearrange("p (j b) -> p b j", j=3)
                    nc.vector.tensor_reduce(red[:], src3, mybir.AxisListType.X,
                                            ALU.add)
                    errt = wk.tile([1, BC], F16, tag="errt")
                    nc.vector.tensor_tensor(errt[:], red[:],
                                            s_negy[:, ts(t - 1, BC)], ALU.add)
                    for h in range(2):
                        mm(b_cur[:, ts(h, BC)], s_wib[:, ts(h, S)], errt[:],
                           start=False, stop=False)

            # ---- [ACT] A_enc tanh (spine) ----
            A_enc = wk.tile([S, BC], F16, tag="A_enc")
            nc.scalar.activation(A_enc[:], enc_cur, AF.Tanh)

            # ---- [PE] fillers while A_enc tanh runs ----
            if not first:
                if FUSE_J:
                    for k in range(3):
                        mm(bj(ns_cur[:, ts(k, BC)]), s_whcat[:, ts(k, S)],
                           gprev, start=False, stop=False)
                else:
                    for k in range(3):
                        for j in range(3):
                            mm(ns_cur[:, ts(k, BC)], s_whcat[:, ts(k, S)],
                               gprev[:, ts(j, BC)], start=False, stop=False)
            if not last:
                xt1 = s_xw[:, ts(t + 1, BC)]
                eg_next = pEnc.tile([S, 512], F32, tag="a_enc")
                enc_next = eg_next[:, 0:BC]
                g_next = eg_next[:, BC:BC + 4]
                mm(enc_next, s_we[:], xt1, start=True, stop=False)
                ns_next = pNs.tile([S, 512], F32, name="a_ns", tag="a_ns")[:, 0:96]
                for k in range(3):
                    mm(ns_next[:, ts(k, BC)], s_wxcat[:, ts(k, S)], xt1,
                       start=(k == 0), stop=False)
            junk(JUNK1)

            # ---- [PE] wit (spine; waits A_enc) ----
            for h in range(2):
                mm(b_cur[:, ts(h, BC)], s_wit[h][:], A_enc[:],
                   start=(first and h == 0), stop=(h == 1))
            # zero-weight matmul: delays a_ns completion (hence A_ns tanh
            # readiness) until after A_enc/wit so the ACT scoreboard runs the
            # spine cst tanh before the off-spine A_ns tanh.
            mm(ns_cur[0:1, 0:BC], s_z1[:], A_enc[:], start=False, stop=True)
            # warm the PE pipe for the head folds during cst tanh
            junk(JUNK2, rhs1=A_enc[:, 0:1])

            # ---- [ACT] cst tanh (spine) ----
            nc.scalar.activation(s_cst[:, 0:2 * BC], b_cur[:], AF.Tanh)

            # ---- [PE] gate head (spine) ----
            mm(g_cur, s_cst[:, 0:4 * BC], s_wfold[:, 0:4],
               start=False, stop=False)
            mm(g_cur, s_cst[:, BC:5 * BC], s_wfold[:, 4:8],
               start=False, stop=True)

            # ---- [PE] fillers for t+1 during the gate phase ----
            if not last:
                b_next = pB.tile([S, 512], F32, name="b_", tag="b_")[:, 0:2 * BC]
                if ERR_PE:
                    # -Wib*y_t rank-1 (err_t = pred_t - y_t for step t+1).
                    # negy is always-ready so it can pass the cst-blocked whc
                    # matmuls in the scoreboard: it must carry the bank's
                    # start=True, not whc00.
                    for h in range(2):
                        mm(b_next[:, ts(h, BC)], s_wib[:, ts(h, S)],
                           s_negy[:, ts(t, BC)], start=(h == 0), stop=False)
                for h in range(2):
                    mm(b_next[:, ts(h, BC)], s_whc[0][h][:], s_cst[:, 0:BC],
                       start=(h == 0 and not ERR_PE), stop=False)
                    mm(b_next[:, ts(h, BC)], s_whc[1][h][:],
                       s_cst[:, BC:2 * BC], start=False, stop=False)
                mm(g_next, s_ones132[:], s_bdext[:], start=False, stop=False)

            # ---- [ACT] exp + th2 (spine), A_ns tanh (off-spine) ----
            e = wk.tile([BC, 3], F32, tag="e")
            nc.scalar.activation(e[:], g_cur[0:BC, 0:3], AF.Exp)

            th2 = wk.tile([BC, 1], F32, tag="th2")
            nc.scalar.activation(th2[:], g_cur[0:BC, 3:4], AF.Tanh)
            A_ns = wk.tile([S, 96], F16, tag="A_ns")
            nc.scalar.activation(A_ns[:], ns_cur[:], AF.Tanh)
            # gate-phase junk: ready once A_ns lands, fills the PE gap between
            # the whc block and gb so gb hits a warm pipe
            junk(JUNK3, rhs1=A_ns[:, 0:1])

            # ---- [DVE] gate algebra (spine) ----
            z = wk.tile([BC, 1], F32, tag="z")
            nc.vector.tensor_reduce(z[:], e[:], mybir.AxisListType.X, ALU.add)
            r0 = wk.tile([BC, 1], F32, tag="r0")
            nc.vector.reciprocal(r0[:], z[:])
            if DS_PACK:
                # ds = [e*r0 - gate_prev | e*r0 + gate_prev]
                ds = wk.tile([BC, 6], F32, tag="ds")
                nc.vector.scalar_tensor_tensor(
                    ds[:].rearrange("p (a b) -> p a b", a=2),
                    e[:].unsqueeze(1).broadcast_to([BC, 2, 3]),
                    r0[:],
                    hgp2[t % 2][:].rearrange("p (a b) -> p a b", a=2),
                    ALU.mult, ALU.add)
                nc.vector.scalar_tensor_tensor(
                    gn[0:BC, 0:3], ds[:, 0:3], th2[:], ds[:, 3:6],
                    ALU.mult, ALU.add)
            else:
                dd = wk.tile([BC, 3], F32, tag="dd")
                nc.vector.scalar_tensor_tensor(
                    dd[:], e[:], r0[:], hgp2[t % 2][:, 0:3],
                    ALU.mult, ALU.add)
                ss = wk.tile([BC, 3], F32, tag="ss")
                nc.vector.scalar_tensor_tensor(
                    ss[:], e[:], r0[:], hgp2[t % 2][:, 3:6],
                    ALU.mult, ALU.add)
                nc.vector.scalar_tensor_tensor(
                    gn[0:BC, 0:3], dd[:], th2[:], ss[:], ALU.mult, ALU.add)

            # ---- [PE] gate transpose+broadcast (spine) ----
            gb = pGB.tile([S, 512], F32, name="gb", tag="gb")[:, 0:96]
            for j in range(3):
                mm(gb[:, ts(j, BC)], gn[:, j:j + 1].broadcast_to([S, S]),
                   s_eye[:], start=(j == 0), stop=(j == 2))
            junk(JUNK4, rhs1=gn[:, 0:1])

            # ---- [DVE] G = A_ns * gateB (spine) + hgp2 for t+1 ----
            nc.vector.tensor_mul(s_gall[:, ts(t, 96)], A_ns[:], gb[:])
            if not last:
                nc.vector.tensor_tensor(
                    hgp2[(t + 1) % 2][:].rearrange("p (a b) -> p a b", a=2),
                    gn[0:BC, 0:3].unsqueeze(1).broadcast_to([BC, 2, 3]),
                    c3, ALU.mult)

            enc_cur, ns_cur, b_cur, g_cur = (
                (None, None, None, None) if last
                else (enc_next, ns_next, b_next, g_next))

        nc.sync.dma_start(out=g127o, in_=s_gall[0:1, :])
        stk.close()
    nc.finalize()
    return nc


# ---------------- host side ----------------

def _pack_inputs(x, y, Wx, Wh, We, Ue, Wi, Whc, Wo, Wd, bd, gate0, Tn=T):
    """Build the 8 per-core input dicts."""
    f16 = np.float16
    F = np.concatenate(
        [Wo[:, :3] @ Wd, 0.5 * Wo[:, 3:4]], axis=1
    ).astype(np.float32)  # [256, 4]
    # permute the S dim so the prediction feature (s=127) sits on partition 0
    # (matmul operands must have base partition 0/32/64)
    perm = np.arange(S)
    perm[[0, S - 1]] = [S - 1, 0]
    Whp = [Wh[j][perm][:, perm] for j in range(3)]
    Wxp = [Wx[j][:, perm] for j in range(3)]

    def padk(a):  # zero-pad contraction dim to 128 rows (FWL eligibility)
        out = np.zeros((S, a.shape[1]), np.float32)
        out[:a.shape[0]] = a
        return out

    eye = np.zeros((S, BC), np.float32)
    eye[0:BC, 0:BC] = 0.5 * np.eye(BC)
    wibr0 = np.zeros((S, H), np.float32)
    wibr0[0] = Wi[E]
    shared = {
        "whcat": np.concatenate(Whp, axis=1).astype(f16),
        "wxcat": padk(np.concatenate(Wxp, axis=1)).astype(f16),
        "ue": Ue[perm, :].astype(f16),
        "we": padk(We).astype(f16),
        "wit0": Wi[0:E, 0:S].astype(f16),
        "wit1": Wi[0:E, S:2 * S].astype(f16),
        "wib": Wi[E:E + 1].astype(f16),
        "wibr0": wibr0.astype(f16),
        "whc00": Whc[0:S, 0:S].astype(f16),
        "whc01": Whc[0:S, S:2 * S].astype(f16),
        "whc10": Whc[S:2 * S, 0:S].astype(f16),
        "whc11": Whc[S:2 * S, S:2 * S].astype(f16),
        "wfold": np.concatenate([F[0:S], F[S:2 * S]], axis=1).astype(f16),
        "bdext": np.concatenate([bd, [0.0]]).reshape(1, 4).astype(f16),
        "eye32": eye.astype(f16),
        "ones132": np.concatenate(
            [np.ones((1, BC)), np.zeros((1, S - BC))], axis=1).astype(f16),
    }
    in_maps = []
    for c in range(NCORES):
        bs = slice(c * BC, (c + 1) * BC)
        xs = x[bs, :Tn]                      # [32, T, 64]
        ys = y[bs, :Tn]                      # [32, T]
        g0 = gate0[bs]                       # [32, 3]
        gf20 = np.zeros((S, 32), np.float32)
        gf20[0:BC, 0:3] = 2.0 * g0
        xwp = np.zeros((S, Tn * BC), np.float32)
        xwp[0:D_IN] = xs.transpose(2, 1, 0).reshape(D_IN, Tn * BC)
        m = dict(shared)
        m["xw"] = xwp.astype(f16)
        m["negy"] = np.ascontiguousarray(
            (-ys.T).reshape(1, Tn * BC)
        ).astype(f16)
        m["gf20"] = gf20.astype(f16)
        in_maps.append(m)
    return in_maps


_PROG_CACHE = {}
LAST_RESULT = {}


def kernel(x, y, Wx, Wh, We, Ue, Wi, Whc, Wo, Wd, bd, gate0):
    from concourse.bass_utils import run_bass_kernel_spmd

    args = [np.asarray(a, dtype=np.float32) for a in
            (x, y, Wx, Wh, We, Ue, Wi, Whc, Wo, Wd, bd, gate0)]
    in_maps = _pack_inputs(*args)
    if "prog" not in _PROG_CACHE:
        _PROG_CACHE["prog"] = build_program(T)
    nc = _PROG_CACHE["prog"]
    trace = bool(int(os.environ.get("TRN_KERNEL_TRACE", "0")))
    res = run_bass_kernel_spmd(
        nc, in_maps, core_ids=list(range(NCORES)), trace=trace
    )
    LAST_RESULT["exec_time_ns"] = res.exec_time_ns
    LAST_RESULT["res"] = res
    preds = np.zeros((B, T), np.float32)
    for c in range(NCORES):
        g127 = res.results[c]["g127"].reshape(T, 3, BC).astype(np.float32)
        preds[c * BC:(c + 1) * BC] = g127.sum(axis=1).T
    return preds


# revision 17
# speedup vs baseline: 1.0940x; 1.0940x over previous
"""Trainium2 Bass kernel for the DiscMaker mkaarma/controller scan.

Math per step t (per batch element b):
    ns    = tanh(x_t @ Wx[j] + kstate @ Wh[j])          j=0..2   [B,3,S]
    enc   = tanh(x_t @ We + kstate @ Ue)                         [B,E]
    cst   = tanh([enc, err] @ Wi + cst @ Whc)                    [B,H]
    out   = cst @ Wo                                             [B,4]
    gate  = softmax(out[:, :3] @ Wd + bd) ; theta = sigmoid(out[:, 3])
    gate  = gate*theta + gate_prev*(1-theta)
    kstate= sum_j gate[:,j] * ns[:,j,:] ; pred = kstate[:,-1] ; err = pred - y_t

Device design (per core, batch shard b=32, feature-on-partition).  The scan is
latency-bound: the serial spine per step is
    gate -> gb broadcast (PE) -> G = ns*gate (DVE) -> Ue/Wi ladder (PE/ACT)
    -> head (PE) -> exp (ACT) -> softmax blend (DVE) -> gate'
so the kernel optimizes the spine:
  - kstate never materialized: carry G[s,(j,b)] = gate[j,b]*ns[s,j,b]; all
    kstate consumers contract G with ONE matmul each whose PSUM out AP repeats
    over j (stride-0) so the 3 j-slices accumulate via has_written bits.
  - err enters the controller through PE only: Wib embedded in row 0 of a
    K=128 weight contracts G (row 0 = pred feature) straight into the
    controller PSUM; -Wib*y_t is a rank-1 matmul off the critical path.
  - gate algebra: exp (no accumulator read), then DVE reduce -> recip -> one
    2-wide packed stt ds=[e*r0 - g | e*r0 + g] -> gn = th2*dd + ss.
    theta via sigmoid(z) = (1+tanh(z/2))/2 keeps the {tanh, exp} ACT table.
  - gate head folded: Wfold = [Wo[:, :3]@Wd, 0.5*Wo[:,3]].
  - software pipelining: whc/negy/input/bdext matmuls for step t+1 are emitted
    into step t's gate-phase PE idle windows; junk matmuls fill the remaining
    PE gaps so the PE p-state stays at 2.4 GHz and the SBUF-access pipeline
    stays primed (first-matmul-after-idle costs ~185ns otherwise).
  - preds come from G[0,:] which is DMA'd out once; host sums over j.
"""

import os
import sys

import numpy as np

sys.path.insert(0, "/opt/trn_rl_repo")

import concourse.bass as bass  # noqa: E402
import concourse.tile as tile  # noqa: E402
from concourse import bacc, mybir  # noqa: E402

F16 = mybir.dt.float16
F32 = mybir.dt.float32
AF = mybir.ActivationFunctionType
ALU = mybir.AluOpType

B, T, D_IN, S, E, H, NOUT = 256, 512, 64, 128, 128, 256, 4
NCORES = 8
BC = B // NCORES  # 32 batch per core


def _junk_cols(env, default):
    v = os.environ.get(env)
    if v is None or v == "":
        return default
    if v.strip() in ("none", "-"):
        return []
    return [int(c) for c in v.split(",") if c]


FUSE_J = bool(int(os.environ.get("TRN_FUSE_J", "1")))
ERR_PE = bool(int(os.environ.get("TRN_ERR_PE", "1")))
DS_PACK = bool(int(os.environ.get("TRN_DS_PACK", "1")))
JUNK1 = _junk_cols("TRN_JUNK1", [])
JUNK2 = _junk_cols("TRN_JUNK2", [128, 128])
JUNK3 = _junk_cols("TRN_JUNK3", [128, 128, 128, 128, 128, 128])
JUNK4 = _junk_cols("TRN_JUNK4", [])


def build_program(T_steps=T):
    nc = bacc.Bacc(
        "TRN2", target_bir_lowering=False, debug=False, enable_asserts=False
    )
    Tn = T_steps

    def din(name, shape, dt=F16):
        return nc.dram_tensor(name, shape, dt, kind="ExternalInput").ap()

    xw = din("xw", [S, Tn * BC])           # x[b,t,d] -> [d, t*32+b], d padded->128
    negy = din("negy", [1, Tn * BC])       # -y[b,t]  -> [1, t*32+b]
    gf20 = din("gf20", [S, 32])            # 2*gate0 in rows 0:32 cols 0:3, else 0
    whcat = din("whcat", [S, 3 * S])       # Wh[j][s_in, s_out] stacked on free
    wxcat = din("wxcat", [S, 3 * S])       # Wx[j] zero-padded rows 64:128
    ue = din("ue", [S, E])
    we = din("we", [S, E])                 # We zero-padded rows 64:128
    wit0 = din("wit0", [E, S])             # Wi[0:128] cols 0:128
    wit1 = din("wit1", [E, S])             # Wi[0:128] cols 128:256
    wib = din("wib", [1, H])               # Wi[128]
    wibr0 = din("wibr0", [S, H])           # Wi[128] embedded in row 0, else 0
    whc00 = din("whc00", [S, S])           # Whc[0:128, 0:128]
    whc01 = din("whc01", [S, S])           # Whc[0:128, 128:256]
    whc10 = din("whc10", [S, S])           # Whc[128:256, 0:128]
    whc11 = din("whc11", [S, S])           # Whc[128:256, 128:256]
    wfold = din("wfold", [S, 8])           # [F[0:128] | F[128:256]], F=[Wo@Wd_ext]
    bdext = din("bdext", [1, 4])           # [bd, 0]
    eye32 = din("eye32", [S, BC])          # 0.5*I_32 in rows 0:32, zeros below
    ones132 = din("ones132", [1, S])       # 1.0 in cols 0:32, 0 elsewhere

    g127o = nc.dram_tensor("g127", [1, Tn * 96], F16, kind="ExternalOutput").ap()

    with tile.TileContext(nc) as tc:
        import contextlib
        stk = contextlib.ExitStack()
        persist = stk.enter_context(tc.tile_pool(name="persist", bufs=1))

        def ptile(shape, dtype, name):
            return persist.tile(shape, dtype, name=name, tag=name)

        # ---- persistent SBUF ----
        s_xw = ptile([S, Tn * BC], F16, "s_xw")
        s_negy = ptile([1, Tn * BC], F16, "s_negy")
        s_whcat = ptile([S, 3 * S], F16, "s_whcat")
        s_wxcat = ptile([S, 3 * S], F16, "s_wxcat")
        s_ue = ptile([S, E], F16, "s_ue")
        s_we = ptile([S, E], F16, "s_we")
        s_wit = [ptile([E, S], F16, "s_wit0"), ptile([E, S], F16, "s_wit1")]
        s_wib = ptile([1, H], F16, "s_wib")
        s_wibr0 = ptile([S, H], F16, "s_wibr0")
        s_whc = [[ptile([S, S], F16, "s_whc00"), ptile([S, S], F16, "s_whc01")],
                 [ptile([S, S], F16, "s_whc10"), ptile([S, S], F16, "s_whc11")]]
        s_wfold = ptile([S, 8], F16, "s_wfold")
        s_bdext = ptile([1, 4], F16, "s_bdext")
        s_eye = ptile([S, BC], F16, "s_eye")
        s_ones132 = ptile([1, S], F16, "s_ones132")
        s_gall = ptile([S, Tn * 96], F16, "s_gall")
        s_cst = ptile([S, 5 * BC], F16, "s_cst")
        gf2a = ptile([S, 32], F16, "gf2a")
        gf2b = ptile([S, 32], F16, "gf2b")
        s_z1 = ptile([S, 1], F16, "s_z1")
        chalf = ptile([BC, 6], F32, "chalf")
        hgp2a = ptile([BC, 6], F32, "hgp2a")
        hgp2b = ptile([BC, 6], F32, "hgp2b")

        for dst, src in [
            (s_xw, xw), (s_negy, negy), (s_whcat, whcat), (s_wxcat, wxcat),
            (s_ue, ue), (s_we, we), (s_wit[0], wit0), (s_wit[1], wit1),
            (s_wib, wib), (s_wibr0, wibr0),
            (s_whc[0][0], whc00), (s_whc[0][1], whc01),
            (s_whc[1][0], whc10), (s_whc[1][1], whc11), (s_wfold, wfold),
            (s_bdext, bdext), (s_eye, eye32), (s_ones132, ones132),
            (gf2a, gf20),
        ]:
            nc.sync.dma_start(out=dst[:], in_=src)
        nc.vector.memset(gf2b[:], 0.0)
        nc.vector.memset(s_z1[:], 0.0)
        nc.vector.memset(s_cst[:, 2 * BC:5 * BC], 0.0)
        nc.vector.memset(chalf[:, 0:3], -0.5)
        nc.vector.memset(chalf[:, 3:6], 0.5)
        # hgp2 = [-gate | +gate] = gf2 * [-0.5 | +0.5]   (gf2 carries 2*gate)
        c3 = chalf[:].rearrange("p (a b) -> p a b", a=2)
        nc.vector.tensor_tensor(
            hgp2a[:].rearrange("p (a b) -> p a b", a=2),
            gf2a[0:BC, 0:3].unsqueeze(1).broadcast_to([BC, 2, 3]),
            c3, ALU.mult)

        # ---- pools ----
        # PSUM is 8 banks of 2KB/partition; zero regions (start=True scope)
        # are bank-sized, so each bank gets exactly ONE start=True per step
        # (the first write); everything else store-on-first-touch/accumulates.
        # The tiny gate-head tile g shares pEnc's bank (cols 32:36).
        pEnc = stk.enter_context(tc.tile_pool(name="pEnc", bufs=2, space="PSUM"))
        pNs = stk.enter_context(tc.tile_pool(name="pNs", bufs=2, space="PSUM"))
        pB = stk.enter_context(tc.tile_pool(name="pB", bufs=2, space="PSUM"))
        pGB = stk.enter_context(tc.tile_pool(name="pGB", bufs=1, space="PSUM"))
        pJ = stk.enter_context(tc.tile_pool(name="pJ", bufs=1, space="PSUM"))
        wk = stk.enter_context(tc.tile_pool(name="wk", bufs=3))

        ts = bass.ts
        gf2 = [gf2a, gf2b]
        hgp2 = [hgp2a, hgp2b]
        mm = nc.tensor.matmul

        jt = pJ.tile([S, 512], F32, name="jt", tag="junk")
        jrot = [0]

        def junk(cols_list, rhs1=None):
            # dependency-gated junk: rhs1 (a [K,1] fp16 SBUF AP) delays
            # readiness until its producer finishes, so the junk fills a
            # specific pipeline gap instead of greedily running early.
            # Output regions rotate across 4 slices of the junk bank so the
            # WAW dependency (sem fires ~173ns after exec) is 4 junks back
            # and never stalls the junk stream.
            for c in cols_list:
                c = min(c, 128)
                off = (jrot[0] % 4) * 128
                jrot[0] += 1
                if rhs1 is None:
                    lhsT, rhs = s_whcat[:, 0:S], s_whcat[:, 0:c]
                else:
                    kp = rhs1.partition_size()
                    lhsT = s_whcat[0:kp, 0:S]
                    rhs = rhs1.broadcast_to([kp, c])
                mm(jt[:, off:off + c], lhsT, rhs,
                   start=True, stop=True, skip_group_check=True)

        def bj(ap):  # [S, n] -> [S, 3, n] with stride-0 j (PSUM accumulate)
            return ap.unsqueeze(1).broadcast_to([ap.shape[0], 3, ap.shape[1]])

        # ---- prologue: step-0 input matmuls ----
        xt0 = s_xw[:, 0:BC]
        eg_cur = pEnc.tile([S, 512], F32, tag="a_enc")
        enc_cur = eg_cur[:, 0:BC]
        g_cur = eg_cur[:, BC:BC + 4]
        mm(enc_cur, s_we[:], xt0, start=True, stop=True)
        mm(g_cur, s_ones132[:], s_bdext[:], start=False, stop=False)
        ns_cur = pNs.tile([S, 512], F32, name="a_ns", tag="a_ns")[:, 0:96]
        for k in range(3):
            mm(ns_cur[:, ts(k, BC)], s_wxcat[:, ts(k, S)], xt0,
               start=(k == 0), stop=(k == 2))
        b_cur = pB.tile([S, 512], F32, name="b_", tag="b_")[:, 0:2 * BC]

        for t in range(Tn):
            first = (t == 0)
            last = (t == Tn - 1)
            gprev = None if first else s_gall[:, ts(t - 1, 96)]
            gn = gf2[(t + 1) % 2]

            # ---- [PE] G-dependent accumulations (spine head) ----
            if not first:
                if FUSE_J:
                    mm(bj(enc_cur), s_ue[:], gprev, start=False, stop=False)
                else:
                    for j in range(3):
                        mm(enc_cur, s_ue[:], gprev[:, ts(j, BC)],
                           start=False, stop=False)
                if ERR_PE:
                    # Wib x pred: row-0-embedded weight contracts G (j-summed)
                    for h in range(2):
                        mm(bj(b_cur[:, ts(h, BC)]), s_wibr0[:, ts(h, S)],
                           gprev, start=False, stop=False)
                else:
                    red = wk.tile([1, BC], F32, tag="red")
                    src3 = gprev[0:1, :].rearrange("p (j b) -> p b j", j=3)
                    nc.vector.tensor_reduce(red[:], src3, mybir.AxisListType.X,
                                            ALU.add)
                    errt = wk.tile([1, BC], F16, tag="errt")
                    nc.vector.tensor_tensor(errt[:], red[:],
                                            s_negy[:, ts(t - 1, BC)], ALU.add)
                    for h in range(2):
                        mm(b_cur[:, ts(h, BC)], s_wib[:, ts(h, S)], errt[:],
                           start=False, stop=False)

            # ---- [ACT] A_enc tanh (spine) ----
            A_enc = wk.tile([S, BC], F16, tag="A_enc")
            nc.scalar.activation(A_enc[:], enc_cur, AF.Tanh)

            # ---- [PE] fillers while A_enc tanh runs ----
            if not first:
                if FUSE_J:
                    for k in range(3):
                        mm(bj(ns_cur[:, ts(k, BC)]), s_whcat[:, ts(k, S)],
                           gprev, start=False, stop=False)
                else:
                    for k in range(3):
                        for j in range(3):
                            mm(ns_cur[:, ts(k, BC)], s_whcat[:, ts(k, S)],
                               gprev[:, ts(j, BC)], start=False, stop=False)
            if not last:
                xt1 = s_xw[:, ts(t + 1, BC)]
                eg_next = pEnc.tile([S, 512], F32, tag="a_enc")
                enc_next = eg_next[:, 0:BC]
                g_next = eg_next[:, BC:BC + 4]
                mm(enc_next, s_we[:], xt1, start=True, stop=False)
                ns_next = pNs.tile([S, 512], F32, name="a_ns", tag="a_ns")[:, 0:96]
                for k in range(3):
                    mm(ns_next[:, ts(k, BC)], s_wxcat[:, ts(k, S)], xt1,
                       start=(k == 0), stop=False)
            junk(JUNK1)

            # ---- [PE] wit (spine; waits A_enc) ----
            for h in range(2):
                mm(b_cur[:, ts(h, BC)], s_wit[h][:], A_enc[:],
                   start=(first and h == 0), stop=(h == 1))
            # zero-weight matmul: delays a_ns completion (hence A_ns tanh
            # readiness) until after A_enc/wit so the ACT scoreboard runs the
            # spine cst tanh before the off-spine A_ns tanh.
            mm(ns_cur[0:1, 0:BC], s_z1[:], A_enc[:], start=False, stop=True)
            # warm the PE pipe for the head folds during cst tanh
            junk(JUNK2, rhs1=A_enc[:, 0:1])

            # ---- [ACT] cst tanh (spine) ----
            nc.scalar.activation(s_cst[:, 0:2 * BC], b_cur[:], AF.Tanh)

            # ---- [PE] gate head (spine) ----
            mm(g_cur, s_cst[:, 0:4 * BC], s_wfold[:, 0:4],
               start=False, stop=False)
            mm(g_cur, s_cst[:, BC:5 * BC], s_wfold[:, 4:8],
               start=False, stop=True)

            # ---- [PE] fillers for t+1 during the gate phase ----
            if not last:
                b_next = pB.tile([S, 512], F32, name="b_", tag="b_")[:, 0:2 * BC]
                if ERR_PE:
                    # -Wib*y_t rank-1 (err_t = pred_t - y_t for step t+1).
                    # negy is always-ready so it can pass the cst-blocked whc
                    # matmuls in the scoreboard: it must carry the bank's
                    # start=True, not whc00.
                    for h in range(2):
                        mm(b_next[:, ts(h, BC)], s_wib[:, ts(h, S)],
                           s_negy[:, ts(t, BC)], start=(h == 0), stop=False)
                for h in range(2):
                    mm(b_next[:, ts(h, BC)], s_whc[0][h][:], s_cst[:, 0:BC],
                       start=(h == 0 and not ERR_PE), stop=False)
                    mm(b_next[:, ts(h, BC)], s_whc[1][h][:],
                       s_cst[:, BC:2 * BC], start=False, stop=False)
                mm(g_next, s_ones132[:], s_bdext[:], start=False, stop=False)

            # ---- [ACT] exp + th2 (spine), A_ns tanh (off-spine) ----
            e = wk.tile([BC, 3], F32, tag="e")
            nc.scalar.activation(e[:], g_cur[0:BC, 0:3], AF.Exp)

            th2 = wk.tile([BC, 1], F32, tag="th2")
            nc.scalar.activation(th2[:], g_cur[0:BC, 3:4], AF.Tanh)
            A_ns = wk.tile([S, 96], F16, tag="A_ns")
            nc.scalar.activation(A_ns[:], ns_cur[:], AF.Tanh)
            # gate-phase junk: ready once A_ns lands, fills the PE gap between
            # the whc block and gb so gb hits a warm pipe
            junk(JUNK3, rhs1=A_ns[:, 0:1])

            # ---- [DVE] gate algebra (spine) ----
            z = wk.tile([BC, 1], F32, tag="z")
            nc.vector.tensor_reduce(z[:], e[:], mybir.AxisListType.X, ALU.add)
            r0 = wk.tile([BC, 1], F32, tag="r0")
            nc.vector.reciprocal(r0[:], z[:])
            if DS_PACK:
                # ds = [e*r0 - gate_prev | e*r0 + gate_prev]
                ds = wk.tile([BC, 6], F32, tag="ds")
                nc.vector.scalar_tensor_tensor(
                    ds[:].rearrange("p (a b) -> p a b", a=2),
                    e[:].unsqueeze(1).broadcast_to([BC, 2, 3]),
                    r0[:],
                    hgp2[t % 2][:].rearrange("p (a b) -> p a b", a=2),
                    ALU.mult, ALU.add)
                nc.vector.scalar_tensor_tensor(
                    gn[0:BC, 0:3], ds[:, 0:3], th2[:], ds[:, 3:6],
                    ALU.mult, ALU.add)
            else:
                dd = wk.tile([BC, 3], F32, tag="dd")
                nc.vector.scalar_tensor_tensor(
                    dd[:], e[:], r0[:], hgp2[t % 2][:, 0:3],
                    ALU.mult, ALU.add)
                ss = wk.tile([BC, 3], F32, tag="ss")
                nc.vector.scalar_tensor_tensor(
                    ss[:], e[:], r0[:], hgp2[t % 2][:, 3:6],
                    ALU.mult, ALU.add)
                nc.vector.scalar_tensor_tensor(
                    gn[0:BC, 0:3], dd[:], th2[:], ss[:], ALU.mult, ALU.add)

            # ---- [PE] gate transpose+broadcast (spine) ----
            gb = pGB.tile([S, 512], F32, name="gb", tag="gb")[:, 0:96]
            for j in range(3):
                mm(gb[:, ts(j, BC)], gn[:, j:j + 1].broadcast_to([S, S]),
                   s_eye[:], start=(j == 0), stop=(j == 2))
            junk(JUNK4, rhs1=gn[:, 0:1])

            # ---- [DVE] G = A_ns * gateB (spine) + hgp2 for t+1 ----
            nc.vector.tensor_mul(s_gall[:, ts(t, 96)], A_ns[:], gb[:])
            if not last:
                nc.vector.tensor_tensor(
                    hgp2[(t + 1) % 2][:].rearrange("p (a b) -> p a b", a=2),
                    gn[0:BC, 0:3].unsqueeze(1).broadcast_to([BC, 2, 3]),
                    c3, ALU.mult)

            enc_cur, ns_cur, b_cur, g_cur = (
                (None, None, None, None) if last
                else (enc_next, ns_next, b_next, g_next))

        nc.sync.dma_start(out=g127o, in_=s_gall[0:1, :])
        stk.close()
    nc.finalize()
    return nc


# ---------------- host side ----------------

def _pack_inputs(x, y, Wx, Wh, We, Ue, Wi, Whc, Wo, Wd, bd, gate0, Tn=T):
    """Build the 8 per-core input dicts."""
    f16 = np.float16
    F = np.concatenate(
        [Wo[:, :3] @ Wd, 0.5 * Wo[:, 3:4]], axis=1
    ).astype(np.float32)  # [256, 4]
    # permute the S dim so the prediction feature (s=127) sits on partition 0
    # (matmul operands must have base partition 0/32/64)
    perm = np.arange(S)
    perm[[0, S - 1]] = [S - 1, 0]
    Whp = [Wh[j][perm][:, perm] for j in range(3)]
    Wxp = [Wx[j][:, perm] for j in range(3)]

    def padk(a):  # zero-pad contraction dim to 128 rows (FWL eligibility)
        out = np.zeros((S, a.shape[1]), np.float32)
        out[:a.shape[0]] = a
        return out

    eye = np.zeros((S, BC), np.float32)
    eye[0:BC, 0:BC] = 0.5 * np.eye(BC)
    wibr0 = np.zeros((S, H), np.float32)
    wibr0[0] = Wi[E]
    shared = {
        "whcat": np.concatenate(Whp, axis=1).astype(f16),
        "wxcat": padk(np.concatenate(Wxp, axis=1)).astype(f16),
        "ue": Ue[perm, :].astype(f16),
        "we": padk(We).astype(f16),
        "wit0": Wi[0:E, 0:S].astype(f16),
        "wit1": Wi[0:E, S:2 * S].astype(f16),
        "wib": Wi[E:E + 1].astype(f16),
        "wibr0": wibr0.astype(f16),
        "whc00": Whc[0:S, 0:S].astype(f16),
        "whc01": Whc[0:S, S:2 * S].astype(f16),
        "whc10": Whc[S:2 * S, 0:S].astype(f16),
        "whc11": Whc[S:2 * S, S:2 * S].astype(f16),
        "wfold": np.concatenate([F[0:S], F[S:2 * S]], axis=1).astype(f16),
        "bdext": np.concatenate([bd, [0.0]]).reshape(1, 4).astype(f16),
        "eye32": eye.astype(f16),
        "ones132": np.concatenate(
            [np.ones((1, BC)), np.zeros((1, S - BC))], axis=1).astype(f16),
    }
    in_maps = []
    for c in range(NCORES):
        bs = slice(c * BC, (c + 1) * BC)
        xs = x[bs, :Tn]                      # [32, T, 64]
        ys = y[bs, :Tn]                      # [32, T]
        g0 = gate0[bs]                       # [32, 3]
        gf20 = np.zeros((S, 32), np.float32)
        gf20[0:BC, 0:3] = 2.0 * g0
        xwp = np.zeros((S, Tn * BC), np.float32)
        xwp[0:D_IN] = xs.transpose(2, 1, 0).reshape(D_IN, Tn * BC)
        m = dict(shared)
        m["xw"] = xwp.astype(f16)
        m["negy"] = np.ascontiguousarray(
            (-ys.T).reshape(1, Tn * BC)
        ).astype(f16)
        m["gf20"] = gf20.astype(f16)
        in_maps.append(m)
    return in_maps


_PROG_CACHE = {}
LAST_RESULT = {}


def kernel(x, y, Wx, Wh, We, Ue, Wi, Whc, Wo, Wd, bd, gate0):
    from concourse.bass_utils import run_bass_kernel_spmd

    args = [np.asarray(a, dtype=np.float32) for a in
            (x, y, Wx, Wh, We, Ue, Wi, Whc, Wo, Wd, bd, gate0)]
    in_maps = _pack_inputs(*args)
    if "prog" not in _PROG_CACHE:
        _PROG_CACHE["prog"] = build_program(T)
    nc = _PROG_CACHE["prog"]
    trace = bool(int(os.environ.get("TRN_KERNEL_TRACE", "0")))
    res = run_bass_kernel_spmd(
        nc, in_maps, core_ids=list(range(NCORES)), trace=trace
    )
    LAST_RESULT["exec_time_ns"] = res.exec_time_ns
    LAST_RESULT["res"] = res
    preds = np.zeros((B, T), np.float32)
    for c in range(NCORES):
        g127 = res.results[c]["g127"].reshape(T, 3, BC).astype(np.float32)
        preds[c * BC:(c + 1) * BC] = g127.sum(axis=1).T
    return preds
